# revision 29
# baseline (speedup 1.0000x reference)
"""Trainium2 Bass kernel for GQA attention block (nn_Attention_6219112644965).

Reference computation (per batch b):
  q = rope(rmsnorm(x @ Wq, q_gamma), cos, sin)   # 16 heads x 128
  k = rope(rmsnorm(x @ Wk, k_gamma), cos, sin)   # 8 kv heads x 128
  v = x @ Wv
  o = softmax(q k^T / sqrt(128)) v               # GQA: q head h uses kv head h//2
  y = o @ Wo

Sharding: 8 cores = 4 batches x 2 head-groups. Core (b, hg) computes q-heads
[hg*8, hg*8+8) / kv-heads [hg*4, hg*4+4) for batch b over the full sequence and
produces a PARTIAL output y_partial = o_hg @ Wo[hg rows]; the host sums the two
partials per batch. No collectives; per-core FLOPs are exactly total/8.

On-chip layout trick: everything is computed head-dim-major (q^T, k^T:
[HD=128 partitions, T free]) so that NO transposes are needed anywhere:
  q^T[h]   = Wq_h^T x^T          (lhsT = Wq slab,  rhs = x^T)
  k^T[kv]  = Wk_kv^T x^T
  v[t,:]   = x^T^T Wv            (lhsT = x^T slice, rhs = Wv)
  s^T      = k_tile^T q^T        (scores transposed: [Tk part, Tq free])
  o^T      = v_tile^T p^T        (accumulated over Tk tiles in PSUM)
  y        = o^T^T Wo            (lhsT = o^T slice, rhs = Wo rows)
Softmax runs over the PARTITION axis of s^T: exp on ScalarE, column sums via
elementwise accumulation (VectorE) + a ones-matmul partition reduction,
reciprocal via exp(-ln(x)) on ScalarE (one table set for the whole kernel),
partition-broadcast on GpSimd. RMSNorm+RoPE run in the head-dim-major layout
with the head dim PERMUTED (even components first) so the rope pair-shuffle
becomes a 64-partition half swap; gamma and the rope sign are folded into
host-precomputed cos/sin tables, and the rmsnorm scale (a per-t row) commutes
with rope and is applied once at the end. softmax max-subtraction is skipped
(scores are O(5) for rmsnorm-ed q,k; fp32 exp is exact there).
"""
import sys

sys.path.insert(0, "/opt/trn_rl_repo")

from contextlib import ExitStack

import ml_dtypes
import numpy as np

import bass_rust
import concourse.bass as bass
import concourse.mybir as mybir
import concourse.tile as tile
from concourse import bacc, hw_specs
from concourse.bass_utils import run_bass_kernel_spmd

F32 = mybir.dt.float32
BF16 = mybir.dt.bfloat16
AF = mybir.ActivationFunctionType

T = 2048          # sequence length
D = 2048          # model dim
HD = 128          # head dim
NQH = 8           # q heads per core
NKV = 4           # kv heads per core
ND = D // 128     # 16 d-tiles
NTT = T // 128    # 16 t-tiles
EPS = 1e-6

_CACHE = {}
LAST_RESULTS = None


class _Bacc(bacc.Bacc):
    """Bacc with Exp pinned to the natural_log_exp_and_others ACT table set.

    The default static func->set assignment maps Exp to `exp_and_others`
    and Ln to `natural_log_exp_and_others`; a kernel alternating Ln and Exp
    then reloads the ACT tables (~2.7us) on every transition.  Hiding `exp`
    from the other sets makes both resolve to the shared set, so the table
    is loaded once for the whole kernel.
    """

    def insert_act_table_loads(self):
        has_activation = any(
            isinstance(i, mybir.InstActivation)
            for b in self.main_func.blocks
            for i in b.instructions
        )
        if not has_activation:
            return
        tables = []
        for name, funcs in hw_specs.get_activation_tables(self.m.arch).items():
            if name != "natural_log_exp_and_others":
                funcs = funcs - {AF.Exp}
            tables.append((name, funcs))
        bass_rust.insert_act_table_loads(self, tables)


def build_module():
    """Build the per-core Bass program (identical on all 8 cores)."""
    nc = _Bacc("TRN2", target_bir_lowering=False, debug=False)

    # ---- DRAM I/O (host-packed so every DMA is contiguous) ----
    xt_d = nc.dram_tensor("xt", [128, ND, T], BF16, kind="ExternalInput")
    wq_d = nc.dram_tensor("wq", [NQH, 128, ND, HD], BF16, kind="ExternalInput")
    wk_d = nc.dram_tensor("wk", [NKV, 128, ND, HD], BF16, kind="ExternalInput")
    wv_d = nc.dram_tensor("wv", [128, ND, NKV * HD], BF16, kind="ExternalInput")
    wo_d = nc.dram_tensor("wo", [128, NQH, D], BF16, kind="ExternalInput")
    cosq_d = nc.dram_tensor("cosq", [128, T], BF16, kind="ExternalInput")
    sinq_d = nc.dram_tensor("sinq", [128, T], BF16, kind="ExternalInput")
    cosk_d = nc.dram_tensor("cosk", [128, T], BF16, kind="ExternalInput")
    sink_d = nc.dram_tensor("sink", [128, T], BF16, kind="ExternalInput")
    y_d = nc.dram_tensor("y", [T, D], F32, kind="ExternalOutput")

    with tile.TileContext(nc) as tc, ExitStack() as top:
        persist = top.enter_context(tc.tile_pool(name="persist", bufs=1))
        qT = persist.tile([128, NQH, T], BF16, tag="qT")     # q^T, rope+norm done
        kT = persist.tile([128, NKV, T], BF16, tag="kT")     # k^T, rope+norm done
        v_sb = persist.tile([128, NTT, NKV * HD], BF16, tag="v")  # v natural
        ones_col = persist.tile([128, 1], BF16, tag="ones")
        nc.vector.memset(ones_col, 1.0)
        zero128 = persist.tile([128, 1], F32, tag="zero128")
        nc.vector.memset(zero128, 0.0)
        zero1 = zero128[0:1, :]
        epsq1 = persist.tile([1, 1], F32, tag="epsq")
        nc.vector.memset(epsq1, float(HD * EPS))
        epsk1 = persist.tile([1, 1], F32, tag="epsk")
        nc.vector.memset(epsk1, float(EPS))

        # ================= phase 1: projections =================
        with ExitStack() as ph1:
            p1 = ph1.enter_context(tc.tile_pool(name="p1", bufs=1))
            xt = p1.tile([128, ND, T], BF16, tag="xt")
            wv_sb = p1.tile([128, ND, NKV * HD], BF16, tag="wv")
            cosq = p1.tile([128, T], BF16, tag="cosq")
            sinq = p1.tile([128, T], BF16, tag="sinq")
            cosk = p1.tile([128, T], BF16, tag="cosk")
            sink = p1.tile([128, T], BF16, tag="sink")

            wslab_p = ph1.enter_context(tc.tile_pool(name="wslab", bufs=2))
            work = ph1.enter_context(tc.tile_pool(name="pwork", bufs=2))
            mwork = ph1.enter_context(tc.tile_pool(name="mwork", bufs=3))
            ps_mm = ph1.enter_context(tc.tile_pool(name="ps_mm", bufs=4, space="PSUM"))
            ps_row = ph1.enter_context(tc.tile_pool(name="ps_row", bufs=2, space="PSUM"))

            def load_wslab(h, w_dram):
                wsl = wslab_p.tile([128, ND, HD], BF16, tag="wsl")
                nc.sync.dma_start(out=wsl[:], in_=w_dram[h])
                return wsl

            wsl_first = wslab_p.tile([128, ND, HD], BF16, tag="wsl")
            nc.sync.dma_start(out=wsl_first[:, 0:8, :], in_=wk_d[0, :, 0:8, :])
            nc.sync.dma_start(out=wsl_first[:, 8:ND, :], in_=wk_d[0, :, 8:ND, :])
            for d in range(ND):
                nc.sync.dma_start(out=xt[:, d, :], in_=xt_d[:, d, :])
            nc.sync.dma_start(out=cosk[:], in_=cosk_d[:])
            nc.sync.dma_start(out=sink[:], in_=sink_d[:])
            nc.sync.dma_start(out=cosq[:], in_=cosq_d[:])
            nc.sync.dma_start(out=sinq[:], in_=sinq_d[:])
            nc.sync.dma_start(out=wv_sb[:], in_=wv_d[:])

            def qk_proj(h, w_dram, out_T, cos_t, sin_t, is_q, wsl=None):
                """One head's projection + rmsnorm + rope, head-dim-major.

                Chunks are processed in pairs sharing each stationary
                LDWEIGHTS (two matmuls per weight tile)."""
                if wsl is None:
                    wsl = load_wslab(h, w_dram)
                pair_ps = []
                for c in range(T // 512):
                    cs = slice(c * 512, (c + 1) * 512)
                    if c % 2 == 0:
                        ps_a = ps_mm.tile([128, 512], F32, tag="mm")
                        ps_b = ps_mm.tile([128, 512], F32, tag="mm")
                        for d in range(ND):
                            nc.tensor.matmul(ps_a, wsl[:, d, :], xt[:, d, cs],
                                             start=(d == 0), stop=(d == ND - 1))
                            nc.tensor.matmul(ps_b, wsl[:, d, :],
                                             xt[:, d, cs.start + 512:cs.stop + 512],
                                             start=(d == 0), stop=(d == ND - 1))
                        pair_ps = [ps_a, ps_b]
                    raw_ps = pair_ps[c % 2]
                    raw_bf = work.tile([128, 512], BF16, tag="raw")
                    nc.scalar.copy(raw_bf, raw_ps)
                    # sum of squares over head dim (partitions) via ones-matmul
                    sq = work.tile([128, 512], BF16, tag="sq")
                    nc.scalar.activation(out=sq, in_=raw_ps, func=AF.Square,
                                         bias=zero128[:, :])
                    ssq = ps_row.tile([1, 512], F32, tag="row")
                    nc.tensor.matmul(ssq, ones_col, sq, start=True, stop=True)
                    # rec = rsqrt(mean+eps) (k) or rsqrt(mean+eps)/sqrt(HD) (q),
                    # via exp(-0.5*ln(scale*ssq+bias)) -- one ACT table set.
                    lnrow = work.tile([1, 512], F32, tag="lnrow")
                    if is_q:
                        nc.scalar.activation(out=lnrow, in_=ssq, func=AF.Ln,
                                             scale=1.0, bias=epsq1[:, :])
                    else:
                        nc.scalar.activation(out=lnrow, in_=ssq, func=AF.Ln,
                                             scale=1.0 / HD, bias=epsk1[:, :])
                    rec = work.tile([1, 512], F32, tag="recrow")
                    nc.scalar.activation(out=rec, in_=lnrow, func=AF.Exp, scale=-0.5,
                                         bias=zero1)
                    bc = work.tile([128, 512], F32, tag="bc")
                    nc.gpsimd.partition_broadcast(bc, rec)
                    # rope: out = raw*cos + swap64(raw)*sin  (gamma folded in tables)
                    m1 = mwork.tile([128, 512], BF16, tag="m1")
                    nc.vector.tensor_mul(m1, raw_bf, cos_t[:, cs])
                    swp = mwork.tile([128, 512], BF16, tag="swp")
                    nc.vector.tensor_copy(swp[0:64, :], raw_bf[64:128, :])
                    nc.vector.tensor_copy(swp[64:128, :], raw_bf[0:64, :])
                    m2 = mwork.tile([128, 512], BF16, tag="m2")
                    nc.vector.tensor_mul(m2, swp, sin_t[:, cs])
                    m3 = mwork.tile([128, 512], BF16, tag="m3")
                    nc.vector.tensor_add(m3, m1, m2)
                    nc.vector.tensor_mul(out_T[:, h, cs], m3, bc)

            for kv in range(NKV):
                qk_proj(kv, wk_d, kT, cosk, sink, is_q=False,
                        wsl=wsl_first if kv == 0 else None)

            for tt in range(NTT):
                v_ps = ps_mm.tile([128, 512], F32, tag="mm")
                ts_ = slice(tt * 128, (tt + 1) * 128)
                for d in range(ND):
                    nc.tensor.matmul(v_ps, xt[:, d, ts_], wv_sb[:, d, :],
                                     start=(d == 0), stop=(d == ND - 1))
                nc.scalar.copy(v_sb[:, tt, :], v_ps)

            for h in range(NQH):
                qk_proj(h, wq_d, qT, cosq, sinq, is_q=True)

        # ================= phase 2: attention + out-projection =================
        with ExitStack() as ph2:
            main2 = ph2.enter_context(tc.tile_pool(name="main2", bufs=1))
            oT = main2.tile([128, NQH, T], BF16, tag="oT")
            wo_sb = main2.tile([128, NQH, D], BF16, tag="wo")
            nc.sync.dma_start(out=wo_sb[:], in_=wo_d[:])

            ysb_p = ph2.enter_context(tc.tile_pool(name="ysb", bufs=3))
            awork = ph2.enter_context(tc.tile_pool(name="awork", bufs=2))
            pwork = ph2.enter_context(tc.tile_pool(name="ppool", bufs=4))
            ps_s = ph2.enter_context(tc.tile_pool(name="ps_s", bufs=2, space="PSUM"))
            # shared pool: attention o-accumulators AND out-projection tiles
            # ([128,1024] slots = 2 banks each; 2 slots + s pool's 4 banks = 8)
            ps_o = ph2.enter_context(tc.tile_pool(name="ps_o", bufs=2, space="PSUM"))

            # attention tq chunks: (start, width). Chunk 0 is wide (no fill
            # work exists yet); later chunks are narrow so almost all
            # out-projection tiles can interleave into ACT-bound windows.
            CHUNKS = [(0, 1024), (1024, 512), (1536, 512)]

            def attn(h, ci):
                kv = h // 2
                c0, W = CHUNKS[ci]
                nhalf = W // 512
                o_ps = ps_o.tile([128, W], F32, tag="o")
                colsum = awork.tile([128, W], BF16, tag="colsum")
                for tk in range(NTT):
                    ks = slice(tk * 128, (tk + 1) * 128)
                    s_ps = ps_s.tile([128, W], F32, tag="s")
                    for j in range(nhalf):
                        nc.tensor.matmul(
                            s_ps[:, j * 512:(j + 1) * 512], kT[:, kv, ks],
                            qT[:, h, c0 + j * 512:c0 + (j + 1) * 512],
                            start=True, stop=True)
                    p_bf = pwork.tile([128, W], BF16, tag="p")
                    nc.scalar.activation(out=p_bf, in_=s_ps, func=AF.Exp,
                                         bias=zero128[:, :])
                    if tk == 0:
                        nc.vector.tensor_copy(colsum, p_bf)
                    else:
                        nc.vector.tensor_add(colsum, colsum, p_bf)
                    vt = v_sb[:, tk, kv * HD:(kv + 1) * HD]
                    for j in range(nhalf):
                        nc.tensor.matmul(
                            o_ps[:, j * 512:(j + 1) * 512], vt,
                            p_bf[:, j * 512:(j + 1) * 512],
                            start=(tk == 0), stop=(tk == NTT - 1))
                # free the o_ps PSUM slot immediately (unnormalized copy);
                # the denominator chain + normalize run off PE's critical path
                oTun = awork.tile([128, W], F32, tag="oTun")
                nc.vector.tensor_copy(oTun, o_ps)
                den = ps_s.tile([128, W], F32, tag="s")  # only row 0 used
                for j in range(nhalf):
                    nc.tensor.matmul(den[0:1, j * 512:(j + 1) * 512], ones_col,
                                     colsum[:, j * 512:(j + 1) * 512],
                                     start=True, stop=True)
                lnr = awork.tile([1, W], F32, tag="lnr")
                nc.scalar.activation(out=lnr, in_=den[0:1, :], func=AF.Ln,
                                     bias=zero1)
                recr = awork.tile([1, W], F32, tag="recr")
                nc.scalar.activation(out=recr, in_=lnr, func=AF.Exp, scale=-1.0,
                                     bias=zero1)
                bc = awork.tile([128, W], F32, tag="abc")
                nc.gpsimd.partition_broadcast(bc, recr)
                nc.vector.tensor_mul(oT[:, h, c0:c0 + W], oTun, bc)

            def outproj(tt, tail=False):
                ts_ = slice(tt * 128, (tt + 1) * 128)
                y_sb = ysb_p.tile([128, D], F32, tag="ysb")
                for np_ in range(2):  # pairs of 512-col chunks share LDWEIGHTS
                    ns0 = slice(np_ * 1024, np_ * 1024 + 512)
                    ns1 = slice(np_ * 1024 + 512, (np_ + 1) * 1024)
                    y_ps = ps_o.tile([128, 1024], F32, tag="o")
                    for h in range(NQH):
                        nc.tensor.matmul(y_ps[:, 0:512], oT[:, h, ts_],
                                         wo_sb[:, h, ns0],
                                         start=(h == 0), stop=(h == NQH - 1))
                        nc.tensor.matmul(y_ps[:, 512:1024], oT[:, h, ts_],
                                         wo_sb[:, h, ns1],
                                         start=(h == 0), stop=(h == NQH - 1))
                    # tail tiles copy on ScalarE (idle there); interleaved ones
                    # on VectorE (ScalarE is exp-bound mid-attention)
                    cp = nc.scalar.copy if tail else nc.vector.tensor_copy
                    cp(y_sb[:, ns0], y_ps[:, 0:512])
                    cp(y_sb[:, ns1], y_ps[:, 512:1024])
                nc.sync.dma_start(out=y_d[ts_, 0:1024], in_=y_sb[:, 0:1024])
                nc.sync.dma_start(out=y_d[ts_, 1024:D], in_=y_sb[:, 1024:D])

            # chunk 0 first (nothing to overlap with it), then narrow chunks
            # with prior chunks' out-projection tiles interleaved so PE slack
            # under the ACT-bound exp windows gets used.
            for h in range(NQH):
                attn(h, 0)
            for h in range(NQH):
                attn(h, 1)
                outproj(h)                       # chunk-0 rows 0..7
            for h in range(NQH):
                attn(h, 2)
                if h % 2 == 1:
                    outproj(8 + h // 2)          # chunk-1 rows 8..11
            for tt in range(12, NTT):
                outproj(tt, tail=True)           # chunk-2 rows 12..15

    nc.compile()
    return nc


def _get_module():
    if "nc" not in _CACHE:
        _CACHE["nc"] = build_module()
    return _CACHE["nc"]


def _pack_inputs(x, cos, sin, Wq, Wk, Wv, Wo, q_gamma, k_gamma):
    """Host-side prep: per-core input dicts with bf16 packed layouts."""
    bf16 = ml_dtypes.bfloat16
    perm = np.concatenate([np.arange(0, HD, 2), np.arange(1, HD, 2)])  # [128]
    partner = np.concatenate([perm[64:], perm[:64]])                   # gamma idx for sin term
    sign = np.concatenate([-np.ones(64), np.ones(64)]).astype(np.float32)

    cosT = np.ascontiguousarray(cos.T)  # [128, T]
    sinT = np.ascontiguousarray(sin.T)

    def tables(gamma):
        c = (cosT[perm] * gamma[perm][:, None]).astype(bf16)
        s = (sinT[perm] * sign[:, None] * gamma[partner][:, None]).astype(bf16)
        return np.ascontiguousarray(c), np.ascontiguousarray(s)

    cosq, sinq = tables(q_gamma.astype(np.float32))
    cosk, sink = tables(k_gamma.astype(np.float32))

    per_hg = []
    for hg in range(2):
        qh = slice(hg * NQH * HD, (hg + 1) * NQH * HD)
        kh = slice(hg * NKV * HD, (hg + 1) * NKV * HD)
        wq = Wq[:, qh].reshape(ND, 128, NQH, HD)[..., perm]
        wq = np.ascontiguousarray(wq.transpose(2, 1, 0, 3)).astype(bf16)
        wk = Wk[:, kh].reshape(ND, 128, NKV, HD)[..., perm]
        wk = np.ascontiguousarray(wk.transpose(2, 1, 0, 3)).astype(bf16)
        wv = Wv[:, kh].reshape(ND, 128, NKV * HD)
        wv = np.ascontiguousarray(wv.transpose(1, 0, 2)).astype(bf16)
        wo = Wo[hg * NQH * HD:(hg + 1) * NQH * HD, :].reshape(NQH, 128, D)
        wo = np.ascontiguousarray(wo.transpose(1, 0, 2)).astype(bf16)
        per_hg.append(dict(wq=wq, wk=wk, wv=wv, wo=wo))

    in_maps = []
    for b in range(4):
        xt = np.ascontiguousarray(
            x[b].T.reshape(ND, 128, T).transpose(1, 0, 2)).astype(bf16)
        for hg in range(2):
            m = dict(xt=xt, cosq=cosq, sinq=sinq, cosk=cosk, sink=sink,
                     **per_hg[hg])
            in_maps.append(m)
    return in_maps


def kernel(x, cos, sin, Wq, Wk, Wv, Wo, q_gamma, k_gamma, **run_kwargs):
    global LAST_RESULTS
    args = [np.asarray(a, dtype=np.float32)
            for a in (x, cos, sin, Wq, Wk, Wv, Wo, q_gamma, k_gamma)]
    nc = _get_module()
    in_maps = _pack_inputs(*args)
    res = run_bass_kernel_spmd(nc, in_maps, core_ids=list(range(8)), **run_kwargs)
    LAST_RESULTS = res
    y = np.empty((4, T, D), dtype=np.float32)
    for b in range(4):
        y[b] = np.asarray(res.results[2 * b]["y"]) + np.asarray(res.results[2 * b + 1]["y"])
    return y


# revision 30
# speedup vs baseline: 1.0347x; 1.0347x over previous
"""Trainium2 Bass kernel for GQA attention block (nn_Attention_6219112644965).

Reference computation (per batch b):
  q = rope(rmsnorm(x @ Wq, q_gamma), cos, sin)   # 16 heads x 128
  k = rope(rmsnorm(x @ Wk, k_gamma), cos, sin)   # 8 kv heads x 128
  v = x @ Wv
  o = softmax(q k^T / sqrt(128)) v               # GQA: q head h uses kv head h//2
  y = o @ Wo

Sharding: 8 cores = 4 batches x 2 head-groups. Core (b, hg) computes q-heads
[hg*8, hg*8+8) / kv-heads [hg*4, hg*4+4) for batch b over the full sequence and
produces a PARTIAL output y_partial = o_hg @ Wo[hg rows]; the host sums the two
partials per batch. No collectives; per-core FLOPs are exactly total/8.

On-chip layout trick: everything is computed head-dim-major (q^T, k^T:
[HD=128 partitions, T free]) so that NO transposes are needed anywhere:
  q^T[h]   = Wq_h^T x^T          (lhsT = Wq slab,  rhs = x^T)
  k^T[kv]  = Wk_kv^T x^T
  v[t,:]   = x^T^T Wv            (lhsT = x^T slice, rhs = Wv)
  s^T      = k_tile^T q^T        (scores transposed: [Tk part, Tq free])
  o^T      = v_tile^T p^T        (accumulated over Tk tiles in PSUM)
  y        = o^T^T Wo            (lhsT = o^T slice, rhs = Wo rows)
Softmax runs over the PARTITION axis of s^T: exp on ScalarE, column sums via
elementwise accumulation (VectorE) + a ones-matmul partition reduction,
reciprocal via exp(-ln(x)) on ScalarE (one table set for the whole kernel),
partition-broadcast on GpSimd. RMSNorm+RoPE run in the head-dim-major layout
with the head dim PERMUTED (even components first) so the rope pair-shuffle
becomes a 64-partition half swap; gamma and the rope sign are folded into
host-precomputed cos/sin tables, and the rmsnorm scale (a per-t row) commutes
with rope and is applied once at the end. softmax max-subtraction is skipped
(scores are O(5) for rmsnorm-ed q,k; fp32 exp is exact there).
"""
import sys

sys.path.insert(0, "/opt/trn_rl_repo")

from contextlib import ExitStack

import ml_dtypes
import numpy as np

import bass_rust
import concourse.bass as bass
import concourse.mybir as mybir
import concourse.tile as tile
from concourse import bacc, hw_specs
from concourse.bass_utils import run_bass_kernel_spmd

F32 = mybir.dt.float32
BF16 = mybir.dt.bfloat16
AF = mybir.ActivationFunctionType

T = 2048          # sequence length
D = 2048          # model dim
HD = 128          # head dim
NQH = 8           # q heads per core
NKV = 4           # kv heads per core
ND = D // 128     # 16 d-tiles
NTT = T // 128    # 16 t-tiles
EPS = 1e-6

_CACHE = {}
LAST_RESULTS = None


class _Bacc(bacc.Bacc):
    """Bacc with Exp pinned to the natural_log_exp_and_others ACT table set.

    The default static func->set assignment maps Exp to `exp_and_others`
    and Ln to `natural_log_exp_and_others`; a kernel alternating Ln and Exp
    then reloads the ACT tables (~2.7us) on every transition.  Hiding `exp`
    from the other sets makes both resolve to the shared set, so the table
    is loaded once for the whole kernel.
    """

    def insert_act_table_loads(self):
        has_activation = any(
            isinstance(i, mybir.InstActivation)
            for b in self.main_func.blocks
            for i in b.instructions
        )
        if not has_activation:
            return
        tables = []
        for name, funcs in hw_specs.get_activation_tables(self.m.arch).items():
            if name != "natural_log_exp_and_others":
                funcs = funcs - {AF.Exp}
            tables.append((name, funcs))
        bass_rust.insert_act_table_loads(self, tables)


def build_module():
    """Build the per-core Bass program (identical on all 8 cores)."""
    nc = _Bacc("TRN2", target_bir_lowering=False, debug=False)

    # ---- DRAM I/O (host-packed so every DMA is contiguous) ----
    xt_d = nc.dram_tensor("xt", [128, ND, T], BF16, kind="ExternalInput")
    wq_d = nc.dram_tensor("wq", [NQH, 128, ND, HD], BF16, kind="ExternalInput")
    wk_d = nc.dram_tensor("wk", [NKV, 128, ND, HD], BF16, kind="ExternalInput")
    wv_d = nc.dram_tensor("wv", [128, ND, NKV * HD], BF16, kind="ExternalInput")
    wo_d = nc.dram_tensor("wo", [128, NQH, D], BF16, kind="ExternalInput")
    cosq_d = nc.dram_tensor("cosq", [128, T], BF16, kind="ExternalInput")
    sinq_d = nc.dram_tensor("sinq", [128, T], BF16, kind="ExternalInput")
    cosk_d = nc.dram_tensor("cosk", [128, T], BF16, kind="ExternalInput")
    sink_d = nc.dram_tensor("sink", [128, T], BF16, kind="ExternalInput")
    y_d = nc.dram_tensor("y", [T, D], F32, kind="ExternalOutput")

    with tile.TileContext(nc) as tc, ExitStack() as top:
        persist = top.enter_context(tc.tile_pool(name="persist", bufs=1))
        qT = persist.tile([128, NQH, T], BF16, tag="qT")     # q^T, rope+norm done
        kT = persist.tile([128, NKV, T], BF16, tag="kT")     # k^T, rope+norm done
        v_sb = persist.tile([128, NTT, NKV * HD], BF16, tag="v")  # v natural
        ones_col = persist.tile([128, 1], BF16, tag="ones")
        nc.vector.memset(ones_col, 1.0)
        zero128 = persist.tile([128, 1], F32, tag="zero128")
        nc.vector.memset(zero128, 0.0)
        zero1 = zero128[0:1, :]
        epsq1 = persist.tile([1, 1], F32, tag="epsq")
        nc.vector.memset(epsq1, float(HD * EPS))
        epsk1 = persist.tile([1, 1], F32, tag="epsk")
        nc.vector.memset(epsk1, float(EPS))

        # ================= phase 1: projections =================
        with ExitStack() as ph1:
            p1 = ph1.enter_context(tc.tile_pool(name="p1", bufs=1))
            xt = p1.tile([128, ND, T], BF16, tag="xt")
            wv_sb = p1.tile([128, ND, NKV * HD], BF16, tag="wv")
            cosq = p1.tile([128, T], BF16, tag="cosq")
            sinq = p1.tile([128, T], BF16, tag="sinq")
            cosk = p1.tile([128, T], BF16, tag="cosk")
            sink = p1.tile([128, T], BF16, tag="sink")

            wslab_p = ph1.enter_context(tc.tile_pool(name="wslab", bufs=2))
            work = ph1.enter_context(tc.tile_pool(name="pwork", bufs=2))
            mwork = ph1.enter_context(tc.tile_pool(name="mwork", bufs=3))
            ps_mm = ph1.enter_context(tc.tile_pool(name="ps_mm", bufs=4, space="PSUM"))
            ps_row = ph1.enter_context(tc.tile_pool(name="ps_row", bufs=2, space="PSUM"))

            def load_wslab(h, w_dram):
                wsl = wslab_p.tile([128, ND, HD], BF16, tag="wsl")
                nc.sync.dma_start(out=wsl[:], in_=w_dram[h])
                return wsl

            wsl_first = wslab_p.tile([128, ND, HD], BF16, tag="wsl")
            nc.sync.dma_start(out=wsl_first[:, 0:8, :], in_=wk_d[0, :, 0:8, :])
            nc.sync.dma_start(out=wsl_first[:, 8:ND, :], in_=wk_d[0, :, 8:ND, :])
            for d in range(ND):
                nc.sync.dma_start(out=xt[:, d, :], in_=xt_d[:, d, :])
            nc.sync.dma_start(out=cosk[:], in_=cosk_d[:])
            nc.sync.dma_start(out=sink[:], in_=sink_d[:])
            nc.sync.dma_start(out=cosq[:], in_=cosq_d[:])
            nc.sync.dma_start(out=sinq[:], in_=sinq_d[:])
            nc.sync.dma_start(out=wv_sb[:], in_=wv_d[:])

            def qk_proj(h, w_dram, out_T, cos_t, sin_t, is_q, wsl=None):
                """One head's projection + rmsnorm + rope, head-dim-major.

                Chunks are processed in pairs sharing each stationary
                LDWEIGHTS (two matmuls per weight tile)."""
                if wsl is None:
                    wsl = load_wslab(h, w_dram)
                pair_ps = []
                for c in range(T // 512):
                    cs = slice(c * 512, (c + 1) * 512)
                    if c % 2 == 0:
                        ps_a = ps_mm.tile([128, 512], F32, tag="mm")
                        ps_b = ps_mm.tile([128, 512], F32, tag="mm")
                        for d in range(ND):
                            nc.tensor.matmul(ps_a, wsl[:, d, :], xt[:, d, cs],
                                             start=(d == 0), stop=(d == ND - 1))
                            nc.tensor.matmul(ps_b, wsl[:, d, :],
                                             xt[:, d, cs.start + 512:cs.stop + 512],
                                             start=(d == 0), stop=(d == ND - 1))
                        pair_ps = [ps_a, ps_b]
                    raw_ps = pair_ps[c % 2]
                    raw_bf = work.tile([128, 512], BF16, tag="raw")
                    nc.scalar.copy(raw_bf, raw_ps)
                    # sum of squares over head dim (partitions) via ones-matmul
                    sq = work.tile([128, 512], BF16, tag="sq")
                    nc.scalar.activation(out=sq, in_=raw_ps, func=AF.Square,
                                         bias=zero128[:, :])
                    ssq = ps_row.tile([1, 512], F32, tag="row")
                    nc.tensor.matmul(ssq, ones_col, sq, start=True, stop=True)
                    # rec = rsqrt(mean+eps) (k) or rsqrt(mean+eps)/sqrt(HD) (q),
                    # via exp(-0.5*ln(scale*ssq+bias)) -- one ACT table set.
                    lnrow = work.tile([1, 512], F32, tag="lnrow")
                    if is_q:
                        nc.scalar.activation(out=lnrow, in_=ssq, func=AF.Ln,
                                             scale=1.0, bias=epsq1[:, :])
                    else:
                        nc.scalar.activation(out=lnrow, in_=ssq, func=AF.Ln,
                                             scale=1.0 / HD, bias=epsk1[:, :])
                    rec = work.tile([1, 512], F32, tag="recrow")
                    nc.scalar.activation(out=rec, in_=lnrow, func=AF.Exp, scale=-0.5,
                                         bias=zero1)
                    bc = work.tile([128, 512], F32, tag="bc")
                    nc.gpsimd.partition_broadcast(bc, rec)
                    # rope: out = raw*cos + swap64(raw)*sin  (gamma folded in tables)
                    m1 = mwork.tile([128, 512], BF16, tag="m1")
                    nc.vector.tensor_mul(m1, raw_bf, cos_t[:, cs])
                    swp = mwork.tile([128, 512], BF16, tag="swp")
                    nc.vector.tensor_copy(swp[0:64, :], raw_bf[64:128, :])
                    nc.vector.tensor_copy(swp[64:128, :], raw_bf[0:64, :])
                    m2 = mwork.tile([128, 512], BF16, tag="m2")
                    nc.vector.tensor_mul(m2, swp, sin_t[:, cs])
                    m3 = mwork.tile([128, 512], BF16, tag="m3")
                    nc.vector.tensor_add(m3, m1, m2)
                    nc.vector.tensor_mul(out_T[:, h, cs], m3, bc)

            for kv in range(NKV):
                qk_proj(kv, wk_d, kT, cosk, sink, is_q=False,
                        wsl=wsl_first if kv == 0 else None)

            for tt in range(NTT):
                v_ps = ps_mm.tile([128, 512], F32, tag="mm")
                ts_ = slice(tt * 128, (tt + 1) * 128)
                for d in range(ND):
                    nc.tensor.matmul(v_ps, xt[:, d, ts_], wv_sb[:, d, :],
                                     start=(d == 0), stop=(d == ND - 1))
                nc.scalar.copy(v_sb[:, tt, :], v_ps)

            for h in range(NQH):
                qk_proj(h, wq_d, qT, cosq, sinq, is_q=True)

        # ================= phase 2: attention + out-projection =================
        with ExitStack() as ph2:
            main2 = ph2.enter_context(tc.tile_pool(name="main2", bufs=1))
            oT = main2.tile([128, NQH, T], BF16, tag="oT")
            wo_sb = main2.tile([128, NQH, D], BF16, tag="wo")
            nc.sync.dma_start(out=wo_sb[:], in_=wo_d[:])

            ysb_p = ph2.enter_context(tc.tile_pool(name="ysb", bufs=3))
            ps_y = ph2.enter_context(tc.tile_pool(name="ps_y", bufs=2, space="PSUM"))
            awork = ph2.enter_context(tc.tile_pool(name="awork", bufs=2))
            pwork = ph2.enter_context(tc.tile_pool(name="ppool", bufs=4))
            ps_s = ph2.enter_context(tc.tile_pool(name="ps_s", bufs=2, space="PSUM"))
            ps_o = ph2.enter_context(tc.tile_pool(name="ps_o", bufs=1, space="PSUM"))

            TQC = 1024  # tq chunk

            def attn(h, c):
                kv = h // 2
                cs0 = slice(c * TQC, c * TQC + 512)
                cs1 = slice(c * TQC + 512, (c + 1) * TQC)
                o_ps = ps_o.tile([128, TQC], F32, tag="o")
                colsum = awork.tile([128, TQC], BF16, tag="colsum")
                for tk in range(NTT):
                    ks = slice(tk * 128, (tk + 1) * 128)
                    s_ps = ps_s.tile([128, TQC], F32, tag="s")
                    nc.tensor.matmul(s_ps[:, 0:512], kT[:, kv, ks], qT[:, h, cs0],
                                     start=True, stop=True)
                    nc.tensor.matmul(s_ps[:, 512:TQC], kT[:, kv, ks], qT[:, h, cs1],
                                     start=True, stop=True)
                    p_bf = pwork.tile([128, TQC], BF16, tag="p")
                    nc.scalar.activation(out=p_bf, in_=s_ps, func=AF.Exp,
                                         bias=zero128[:, :])
                    if tk == 0:
                        nc.vector.tensor_copy(colsum, p_bf)
                    else:
                        nc.vector.tensor_add(colsum, colsum, p_bf)
                    vt = v_sb[:, tk, kv * HD:(kv + 1) * HD]
                    nc.tensor.matmul(o_ps[:, 0:512], vt, p_bf[:, 0:512],
                                     start=(tk == 0), stop=(tk == NTT - 1))
                    nc.tensor.matmul(o_ps[:, 512:TQC], vt, p_bf[:, 512:TQC],
                                     start=(tk == 0), stop=(tk == NTT - 1))
                # free the o_ps PSUM slot immediately (unnormalized copy);
                # the denominator chain + normalize run off PE's critical path
                oTun = awork.tile([128, TQC], F32, tag="oTun")
                nc.vector.tensor_copy(oTun, o_ps)
                den = ps_s.tile([128, TQC], F32, tag="s")  # only row 0 used
                nc.tensor.matmul(den[0:1, 0:512], ones_col, colsum[:, 0:512],
                                 start=True, stop=True)
                nc.tensor.matmul(den[0:1, 512:TQC], ones_col, colsum[:, 512:TQC],
                                 start=True, stop=True)
                lnr = awork.tile([1, TQC], F32, tag="lnr")
                nc.scalar.activation(out=lnr, in_=den[0:1, :], func=AF.Ln,
                                     bias=zero1)
                recr = awork.tile([1, TQC], F32, tag="recr")
                nc.scalar.activation(out=recr, in_=lnr, func=AF.Exp, scale=-1.0,
                                     bias=zero1)
                bc = awork.tile([128, TQC], F32, tag="abc")
                nc.gpsimd.partition_broadcast(bc, recr)
                nc.vector.tensor_mul(oT[:, h, c * TQC:(c + 1) * TQC], oTun, bc)

            def outproj(tt, tail=False):
                ts_ = slice(tt * 128, (tt + 1) * 128)
                y_sb = ysb_p.tile([128, D], F32, tag="ysb")
                for np_ in range(2):  # pairs of 512-col chunks share LDWEIGHTS
                    ns0 = slice(np_ * 1024, np_ * 1024 + 512)
                    ns1 = slice(np_ * 1024 + 512, (np_ + 1) * 1024)
                    y_ps0 = ps_y.tile([128, 512], F32, tag="y")
                    y_ps1 = ps_y.tile([128, 512], F32, tag="y")
                    for h in range(NQH):
                        nc.tensor.matmul(y_ps0, oT[:, h, ts_], wo_sb[:, h, ns0],
                                         start=(h == 0), stop=(h == NQH - 1))
                        nc.tensor.matmul(y_ps1, oT[:, h, ts_], wo_sb[:, h, ns1],
                                         start=(h == 0), stop=(h == NQH - 1))
                    # tail tiles copy on ScalarE (idle there); interleaved ones
                    # on VectorE (ScalarE is exp-bound mid-attention)
                    cp = nc.scalar.copy if tail else nc.vector.tensor_copy
                    cp(y_sb[:, ns0], y_ps0)
                    cp(y_sb[:, ns1], y_ps1)
                nc.sync.dma_start(out=y_d[ts_, 0:1024], in_=y_sb[:, 0:1024])
                nc.sync.dma_start(out=y_d[ts_, 1024:D], in_=y_sb[:, 1024:D])

            # attention chunk 0, then chunk 1 with chunk-0's out-projection
            # interleaved so its matmuls fill PE slack while ACT runs exps.
            for h in range(NQH):
                attn(h, 0)
            for h in range(NQH):
                attn(h, 1)
                outproj(h)          # chunk-0 rows 0..7
            for tt in range(8, NTT):
                outproj(tt, tail=True)  # chunk-1 rows

    nc.compile()
    return nc


def _get_module():
    if "nc" not in _CACHE:
        _CACHE["nc"] = build_module()
    return _CACHE["nc"]


def _pack_inputs(x, cos, sin, Wq, Wk, Wv, Wo, q_gamma, k_gamma):
    """Host-side prep: per-core input dicts with bf16 packed layouts."""
    bf16 = ml_dtypes.bfloat16
    perm = np.concatenate([np.arange(0, HD, 2), np.arange(1, HD, 2)])  # [128]
    partner = np.concatenate([perm[64:], perm[:64]])                   # gamma idx for sin term
    sign = np.concatenate([-np.ones(64), np.ones(64)]).astype(np.float32)

    cosT = np.ascontiguousarray(cos.T)  # [128, T]
    sinT = np.ascontiguousarray(sin.T)

    def tables(gamma):
        c = (cosT[perm] * gamma[perm][:, None]).astype(bf16)
        s = (sinT[perm] * sign[:, None] * gamma[partner][:, None]).astype(bf16)
        return np.ascontiguousarray(c), np.ascontiguousarray(s)

    cosq, sinq = tables(q_gamma.astype(np.float32))
    cosk, sink = tables(k_gamma.astype(np.float32))

    per_hg = []
    for hg in range(2):
        qh = slice(hg * NQH * HD, (hg + 1) * NQH * HD)
        kh = slice(hg * NKV * HD, (hg + 1) * NKV * HD)
        wq = Wq[:, qh].reshape(ND, 128, NQH, HD)[..., perm]
        wq = np.ascontiguousarray(wq.transpose(2, 1, 0, 3)).astype(bf16)
        wk = Wk[:, kh].reshape(ND, 128, NKV, HD)[..., perm]
        wk = np.ascontiguousarray(wk.transpose(2, 1, 0, 3)).astype(bf16)
        wv = Wv[:, kh].reshape(ND, 128, NKV * HD)
        wv = np.ascontiguousarray(wv.transpose(1, 0, 2)).astype(bf16)
        wo = Wo[hg * NQH * HD:(hg + 1) * NQH * HD, :].reshape(NQH, 128, D)
        wo = np.ascontiguousarray(wo.transpose(1, 0, 2)).astype(bf16)
        per_hg.append(dict(wq=wq, wk=wk, wv=wv, wo=wo))

    in_maps = []
    for b in range(4):
        xt = np.ascontiguousarray(
            x[b].T.reshape(ND, 128, T).transpose(1, 0, 2)).astype(bf16)
        for hg in range(2):
            m = dict(xt=xt, cosq=cosq, sinq=sinq, cosk=cosk, sink=sink,
                     **per_hg[hg])
            in_maps.append(m)
    return in_maps


def kernel(x, cos, sin, Wq, Wk, Wv, Wo, q_gamma, k_gamma, **run_kwargs):
    global LAST_RESULTS
    args = [np.asarray(a, dtype=np.float32)
            for a in (x, cos, sin, Wq, Wk, Wv, Wo, q_gamma, k_gamma)]
    nc = _get_module()
    in_maps = _pack_inputs(*args)
    res = run_bass_kernel_spmd(nc, in_maps, core_ids=list(range(8)), **run_kwargs)
    LAST_RESULTS = res
    y = np.empty((4, T, D), dtype=np.float32)
    for b in range(4):
        y[b] = np.asarray(res.results[2 * b]["y"]) + np.asarray(res.results[2 * b + 1]["y"])
    return y


# revision 32
# speedup vs baseline: 1.0353x; 1.0006x over previous
"""Trainium2 Bass kernel for GQA attention block (nn_Attention_6219112644965).

Reference computation (per batch b):
  q = rope(rmsnorm(x @ Wq, q_gamma), cos, sin)   # 16 heads x 128
  k = rope(rmsnorm(x @ Wk, k_gamma), cos, sin)   # 8 kv heads x 128
  v = x @ Wv
  o = softmax(q k^T / sqrt(128)) v               # GQA: q head h uses kv head h//2
  y = o @ Wo

Sharding: 8 cores = 4 batches x 2 head-groups. Core (b, hg) computes q-heads
[hg*8, hg*8+8) / kv-heads [hg*4, hg*4+4) for batch b over the full sequence and
produces a PARTIAL output y_partial = o_hg @ Wo[hg rows]; the host sums the two
partials per batch. No collectives; per-core FLOPs are exactly total/8.

On-chip layout trick: everything is computed head-dim-major (q^T, k^T:
[HD=128 partitions, T free]) so that NO transposes are needed anywhere:
  q^T[h]   = Wq_h^T x^T          (lhsT = Wq slab,  rhs = x^T)
  k^T[kv]  = Wk_kv^T x^T
  v[t,:]   = x^T^T Wv            (lhsT = x^T slice, rhs = Wv)
  s^T      = k_tile^T q^T        (scores transposed: [Tk part, Tq free])
  o^T      = v_tile^T p^T        (accumulated over Tk tiles in PSUM)
  y        = o^T^T Wo            (lhsT = o^T slice, rhs = Wo rows)
Softmax runs over the PARTITION axis of s^T: exp on ScalarE, column sums via
elementwise accumulation (VectorE) + a ones-matmul partition reduction,
reciprocal via exp(-ln(x)) on ScalarE (one table set for the whole kernel),
partition-broadcast on GpSimd. RMSNorm+RoPE run in the head-dim-major layout
with the head dim PERMUTED (even components first) so the rope pair-shuffle
becomes a 64-partition half swap; gamma and the rope sign are folded into
host-precomputed cos/sin tables, and the rmsnorm scale (a per-t row) commutes
with rope and is applied once at the end. softmax max-subtraction is skipped
(scores are O(5) for rmsnorm-ed q,k; fp32 exp is exact there).
"""
import sys

sys.path.insert(0, "/opt/trn_rl_repo")

from contextlib import ExitStack

import ml_dtypes
import numpy as np

import bass_rust
import concourse.bass as bass
import concourse.mybir as mybir
import concourse.tile as tile
from concourse import bacc, hw_specs
from concourse.bass_utils import run_bass_kernel_spmd

F32 = mybir.dt.float32
BF16 = mybir.dt.bfloat16
AF = mybir.ActivationFunctionType

T = 2048          # sequence length
D = 2048          # model dim
HD = 128          # head dim
NQH = 8           # q heads per core
NKV = 4           # kv heads per core
ND = D // 128     # 16 d-tiles
NTT = T // 128    # 16 t-tiles
EPS = 1e-6

_CACHE = {}
LAST_RESULTS = None


class _Bacc(bacc.Bacc):
    """Bacc with Exp pinned to the natural_log_exp_and_others ACT table set.

    The default static func->set assignment maps Exp to `exp_and_others`
    and Ln to `natural_log_exp_and_others`; a kernel alternating Ln and Exp
    then reloads the ACT tables (~2.7us) on every transition.  Hiding `exp`
    from the other sets makes both resolve to the shared set, so the table
    is loaded once for the whole kernel.
    """

    def insert_act_table_loads(self):
        has_activation = any(
            isinstance(i, mybir.InstActivation)
            for b in self.main_func.blocks
            for i in b.instructions
        )
        if not has_activation:
            return
        tables = []
        for name, funcs in hw_specs.get_activation_tables(self.m.arch).items():
            if name != "natural_log_exp_and_others":
                funcs = funcs - {AF.Exp}
            tables.append((name, funcs))
        bass_rust.insert_act_table_loads(self, tables)


def build_module():
    """Build the per-core Bass program (identical on all 8 cores)."""
    nc = _Bacc("TRN2", target_bir_lowering=False, debug=False)

    # ---- DRAM I/O (host-packed so every DMA is contiguous) ----
    xt_d = nc.dram_tensor("xt", [128, ND, T], BF16, kind="ExternalInput")
    wq_d = nc.dram_tensor("wq", [NQH, 128, ND, HD], BF16, kind="ExternalInput")
    wk_d = nc.dram_tensor("wk", [NKV, 128, ND, HD], BF16, kind="ExternalInput")
    wv_d = nc.dram_tensor("wv", [128, ND, NKV * HD], BF16, kind="ExternalInput")
    wo_d = nc.dram_tensor("wo", [128, NQH, D], BF16, kind="ExternalInput")
    cosq_d = nc.dram_tensor("cosq", [128, T], BF16, kind="ExternalInput")
    sinq_d = nc.dram_tensor("sinq", [128, T], BF16, kind="ExternalInput")
    cosk_d = nc.dram_tensor("cosk", [128, T], BF16, kind="ExternalInput")
    sink_d = nc.dram_tensor("sink", [128, T], BF16, kind="ExternalInput")
    y_d = nc.dram_tensor("y", [T, D], F32, kind="ExternalOutput")

    with tile.TileContext(nc) as tc, ExitStack() as top:
        persist = top.enter_context(tc.tile_pool(name="persist", bufs=1))
        qT = persist.tile([128, NQH, T], BF16, tag="qT")     # q^T, rope+norm done
        kT = persist.tile([128, NKV, T], BF16, tag="kT")     # k^T, rope+norm done
        v_sb = persist.tile([128, NTT, NKV * HD], BF16, tag="v")  # v natural
        ones_col = persist.tile([128, 1], BF16, tag="ones")
        nc.vector.memset(ones_col, 1.0)
        zero128 = persist.tile([128, 1], F32, tag="zero128")
        nc.vector.memset(zero128, 0.0)
        zero1 = zero128[0:1, :]
        epsq1 = persist.tile([1, 1], F32, tag="epsq")
        nc.vector.memset(epsq1, float(HD * EPS))
        epsk1 = persist.tile([1, 1], F32, tag="epsk")
        nc.vector.memset(epsk1, float(EPS))

        # ================= phase 1: projections =================
        with ExitStack() as ph1:
            p1 = ph1.enter_context(tc.tile_pool(name="p1", bufs=1))
            xt = p1.tile([128, ND, T], BF16, tag="xt")
            wv_sb = p1.tile([128, ND, NKV * HD], BF16, tag="wv")
            cosq = p1.tile([128, T], BF16, tag="cosq")
            sinq = p1.tile([128, T], BF16, tag="sinq")
            cosk = p1.tile([128, T], BF16, tag="cosk")
            sink = p1.tile([128, T], BF16, tag="sink")

            wslab_p = ph1.enter_context(tc.tile_pool(name="wslab", bufs=2))
            work = ph1.enter_context(tc.tile_pool(name="pwork", bufs=2))
            mwork = ph1.enter_context(tc.tile_pool(name="mwork", bufs=3))
            ps_mm = ph1.enter_context(tc.tile_pool(name="ps_mm", bufs=4, space="PSUM"))
            ps_row = ph1.enter_context(tc.tile_pool(name="ps_row", bufs=2, space="PSUM"))

            def load_wslab(h, w_dram):
                wsl = wslab_p.tile([128, ND, HD], BF16, tag="wsl")
                nc.sync.dma_start(out=wsl[:], in_=w_dram[h])
                return wsl

            wsl_first = wslab_p.tile([128, ND, HD], BF16, tag="wsl")
            nc.sync.dma_start(out=wsl_first[:, 0:8, :], in_=wk_d[0, :, 0:8, :])
            nc.sync.dma_start(out=wsl_first[:, 8:ND, :], in_=wk_d[0, :, 8:ND, :])
            for d in range(ND):
                nc.sync.dma_start(out=xt[:, d, :], in_=xt_d[:, d, :])
            nc.sync.dma_start(out=cosk[:], in_=cosk_d[:])
            nc.sync.dma_start(out=sink[:], in_=sink_d[:])
            nc.sync.dma_start(out=cosq[:], in_=cosq_d[:])
            nc.sync.dma_start(out=sinq[:], in_=sinq_d[:])
            nc.sync.dma_start(out=wv_sb[:], in_=wv_d[:])

            def qk_proj(h, w_dram, out_T, cos_t, sin_t, is_q, wsl=None):
                """One head's projection + rmsnorm + rope, head-dim-major.

                Chunks are processed in pairs sharing each stationary
                LDWEIGHTS (two matmuls per weight tile)."""
                if wsl is None:
                    wsl = load_wslab(h, w_dram)
                pair_ps = []
                for c in range(T // 512):
                    cs = slice(c * 512, (c + 1) * 512)
                    if c % 2 == 0:
                        ps_a = ps_mm.tile([128, 512], F32, tag="mm")
                        ps_b = ps_mm.tile([128, 512], F32, tag="mm")
                        for d in range(ND):
                            nc.tensor.matmul(ps_a, wsl[:, d, :], xt[:, d, cs],
                                             start=(d == 0), stop=(d == ND - 1))
                            nc.tensor.matmul(ps_b, wsl[:, d, :],
                                             xt[:, d, cs.start + 512:cs.stop + 512],
                                             start=(d == 0), stop=(d == ND - 1))
                        pair_ps = [ps_a, ps_b]
                    raw_ps = pair_ps[c % 2]
                    raw_bf = work.tile([128, 512], BF16, tag="raw")
                    nc.scalar.copy(raw_bf, raw_ps)
                    # sum of squares over head dim (partitions) via ones-matmul
                    sq = work.tile([128, 512], BF16, tag="sq")
                    nc.scalar.activation(out=sq, in_=raw_ps, func=AF.Square,
                                         bias=zero128[:, :])
                    ssq = ps_row.tile([1, 512], F32, tag="row")
                    nc.tensor.matmul(ssq, ones_col, sq, start=True, stop=True)
                    # rec = rsqrt(mean+eps) (k) or rsqrt(mean+eps)/sqrt(HD) (q),
                    # via exp(-0.5*ln(scale*ssq+bias)) -- one ACT table set.
                    lnrow = work.tile([1, 512], F32, tag="lnrow")
                    if is_q:
                        nc.scalar.activation(out=lnrow, in_=ssq, func=AF.Ln,
                                             scale=1.0, bias=epsq1[:, :])
                    else:
                        nc.scalar.activation(out=lnrow, in_=ssq, func=AF.Ln,
                                             scale=1.0 / HD, bias=epsk1[:, :])
                    rec = work.tile([1, 512], F32, tag="recrow")
                    nc.scalar.activation(out=rec, in_=lnrow, func=AF.Exp, scale=-0.5,
                                         bias=zero1)
                    bc = work.tile([128, 512], F32, tag="bc")
                    nc.gpsimd.partition_broadcast(bc, rec)
                    # rope: out = raw*cos + swap64(raw)*sin  (gamma folded in tables)
                    m1 = mwork.tile([128, 512], BF16, tag="m1")
                    nc.vector.tensor_mul(m1, raw_bf, cos_t[:, cs])
                    swp = mwork.tile([128, 512], BF16, tag="swp")
                    nc.vector.tensor_copy(swp[0:64, :], raw_bf[64:128, :])
                    nc.vector.tensor_copy(swp[64:128, :], raw_bf[0:64, :])
                    m2 = mwork.tile([128, 512], BF16, tag="m2")
                    nc.vector.tensor_mul(m2, swp, sin_t[:, cs])
                    m3 = mwork.tile([128, 512], BF16, tag="m3")
                    nc.vector.tensor_add(m3, m1, m2)
                    nc.vector.tensor_mul(out_T[:, h, cs], m3, bc)

            for kv in range(NKV):
                qk_proj(kv, wk_d, kT, cosk, sink, is_q=False,
                        wsl=wsl_first if kv == 0 else None)

            for tt in range(NTT):
                v_ps = ps_mm.tile([128, 512], F32, tag="mm")
                ts_ = slice(tt * 128, (tt + 1) * 128)
                for d in range(ND):
                    nc.tensor.matmul(v_ps, xt[:, d, ts_], wv_sb[:, d, :],
                                     start=(d == 0), stop=(d == ND - 1))
                nc.scalar.copy(v_sb[:, tt, :], v_ps)

            for h in range(NQH):
                qk_proj(h, wq_d, qT, cosq, sinq, is_q=True)

        # ================= phase 2: attention + out-projection =================
        with ExitStack() as ph2:
            main2 = ph2.enter_context(tc.tile_pool(name="main2", bufs=1))
            oT = main2.tile([128, NQH, T], BF16, tag="oT")
            wo_sb = main2.tile([128, NQH, D], BF16, tag="wo")
            nc.sync.dma_start(out=wo_sb[:], in_=wo_d[:])

            ysb_p = ph2.enter_context(tc.tile_pool(name="ysb", bufs=4))
            ps_y = ph2.enter_context(tc.tile_pool(name="ps_y", bufs=2, space="PSUM"))
            awork = ph2.enter_context(tc.tile_pool(name="awork", bufs=2))
            pwork = ph2.enter_context(tc.tile_pool(name="ppool", bufs=5))
            ps_s = ph2.enter_context(tc.tile_pool(name="ps_s", bufs=2, space="PSUM"))
            ps_o = ph2.enter_context(tc.tile_pool(name="ps_o", bufs=1, space="PSUM"))

            TQC = 1024  # tq chunk

            def attn(h, c):
                kv = h // 2
                cs0 = slice(c * TQC, c * TQC + 512)
                cs1 = slice(c * TQC + 512, (c + 1) * TQC)
                o_ps = ps_o.tile([128, TQC], F32, tag="o")
                colsum = awork.tile([128, TQC], BF16, tag="colsum")
                for tk in range(NTT):
                    ks = slice(tk * 128, (tk + 1) * 128)
                    s_ps = ps_s.tile([128, TQC], F32, tag="s")
                    nc.tensor.matmul(s_ps[:, 0:512], kT[:, kv, ks], qT[:, h, cs0],
                                     start=True, stop=True)
                    nc.tensor.matmul(s_ps[:, 512:TQC], kT[:, kv, ks], qT[:, h, cs1],
                                     start=True, stop=True)
                    p_bf = pwork.tile([128, TQC], BF16, tag="p")
                    nc.scalar.activation(out=p_bf, in_=s_ps, func=AF.Exp,
                                         bias=zero128[:, :])
                    if tk == 0:
                        nc.vector.tensor_copy(colsum, p_bf)
                    else:
                        nc.vector.tensor_add(colsum, colsum, p_bf)
                    vt = v_sb[:, tk, kv * HD:(kv + 1) * HD]
                    nc.tensor.matmul(o_ps[:, 0:512], vt, p_bf[:, 0:512],
                                     start=(tk == 0), stop=(tk == NTT - 1))
                    nc.tensor.matmul(o_ps[:, 512:TQC], vt, p_bf[:, 512:TQC],
                                     start=(tk == 0), stop=(tk == NTT - 1))
                # free the o_ps PSUM slot immediately (unnormalized copy);
                # the denominator chain + normalize run off PE's critical path
                oTun = awork.tile([128, TQC], F32, tag="oTun")
                nc.vector.tensor_copy(oTun, o_ps)
                den = ps_s.tile([128, TQC], F32, tag="s")  # only row 0 used
                nc.tensor.matmul(den[0:1, 0:512], ones_col, colsum[:, 0:512],
                                 start=True, stop=True)
                nc.tensor.matmul(den[0:1, 512:TQC], ones_col, colsum[:, 512:TQC],
                                 start=True, stop=True)
                # reciprocal on VectorE (custom uop, ~51 ULP) — keeps the
                # denominator chain off exp-bound ScalarE entirely
                recr = awork.tile([1, TQC], F32, tag="recr")
                nc.vector.reciprocal_approx_fast(out=recr, in_=den[0:1, :])
                bc = awork.tile([128, TQC], F32, tag="abc")
                nc.gpsimd.partition_broadcast(bc, recr)
                nc.vector.tensor_mul(oT[:, h, c * TQC:(c + 1) * TQC], oTun, bc)

            def outproj(tt, tail=False):
                ts_ = slice(tt * 128, (tt + 1) * 128)
                y_sb = ysb_p.tile([128, D], F32, tag="ysb")
                for np_ in range(2):  # pairs of 512-col chunks share LDWEIGHTS
                    ns0 = slice(np_ * 1024, np_ * 1024 + 512)
                    ns1 = slice(np_ * 1024 + 512, (np_ + 1) * 1024)
                    y_ps0 = ps_y.tile([128, 512], F32, tag="y")
                    y_ps1 = ps_y.tile([128, 512], F32, tag="y")
                    for h in range(NQH):
                        nc.tensor.matmul(y_ps0, oT[:, h, ts_], wo_sb[:, h, ns0],
                                         start=(h == 0), stop=(h == NQH - 1))
                        nc.tensor.matmul(y_ps1, oT[:, h, ts_], wo_sb[:, h, ns1],
                                         start=(h == 0), stop=(h == NQH - 1))
                    # tail tiles copy on ScalarE (idle there); interleaved ones
                    # on VectorE (ScalarE is exp-bound mid-attention)
                    cp = nc.scalar.copy if tail else nc.vector.tensor_copy
                    cp(y_sb[:, ns0], y_ps0)
                    cp(y_sb[:, ns1], y_ps1)
                nc.sync.dma_start(out=y_d[ts_, 0:1024], in_=y_sb[:, 0:1024])
                nc.sync.dma_start(out=y_d[ts_, 1024:D], in_=y_sb[:, 1024:D])

            # attention chunk 0, then chunk 1 with chunk-0's out-projection
            # interleaved so its matmuls fill PE slack while ACT runs exps.
            for h in range(NQH):
                attn(h, 0)
            for h in range(NQH):
                attn(h, 1)
                outproj(h)          # chunk-0 rows 0..7
            for tt in range(8, NTT):
                outproj(tt, tail=True)  # chunk-1 rows

    nc.compile()
    return nc


def _get_module():
    if "nc" not in _CACHE:
        _CACHE["nc"] = build_module()
    return _CACHE["nc"]


def _pack_inputs(x, cos, sin, Wq, Wk, Wv, Wo, q_gamma, k_gamma):
    """Host-side prep: per-core input dicts with bf16 packed layouts."""
    bf16 = ml_dtypes.bfloat16
    perm = np.concatenate([np.arange(0, HD, 2), np.arange(1, HD, 2)])  # [128]
    partner = np.concatenate([perm[64:], perm[:64]])                   # gamma idx for sin term
    sign = np.concatenate([-np.ones(64), np.ones(64)]).astype(np.float32)

    cosT = np.ascontiguousarray(cos.T)  # [128, T]
    sinT = np.ascontiguousarray(sin.T)

    def tables(gamma):
        c = (cosT[perm] * gamma[perm][:, None]).astype(bf16)
        s = (sinT[perm] * sign[:, None] * gamma[partner][:, None]).astype(bf16)
        return np.ascontiguousarray(c), np.ascontiguousarray(s)

    cosq, sinq = tables(q_gamma.astype(np.float32))
    cosk, sink = tables(k_gamma.astype(np.float32))

    per_hg = []
    for hg in range(2):
        qh = slice(hg * NQH * HD, (hg + 1) * NQH * HD)
        kh = slice(hg * NKV * HD, (hg + 1) * NKV * HD)
        wq = Wq[:, qh].reshape(ND, 128, NQH, HD)[..., perm]
        wq = np.ascontiguousarray(wq.transpose(2, 1, 0, 3)).astype(bf16)
        wk = Wk[:, kh].reshape(ND, 128, NKV, HD)[..., perm]
        wk = np.ascontiguousarray(wk.transpose(2, 1, 0, 3)).astype(bf16)
        wv = Wv[:, kh].reshape(ND, 128, NKV * HD)
        wv = np.ascontiguousarray(wv.transpose(1, 0, 2)).astype(bf16)
        wo = Wo[hg * NQH * HD:(hg + 1) * NQH * HD, :].reshape(NQH, 128, D)
        wo = np.ascontiguousarray(wo.transpose(1, 0, 2)).astype(bf16)
        per_hg.append(dict(wq=wq, wk=wk, wv=wv, wo=wo))

    in_maps = []
    for b in range(4):
        xt = np.ascontiguousarray(
            x[b].T.reshape(ND, 128, T).transpose(1, 0, 2)).astype(bf16)
        for hg in range(2):
            m = dict(xt=xt, cosq=cosq, sinq=sinq, cosk=cosk, sink=sink,
                     **per_hg[hg])
            in_maps.append(m)
    return in_maps


def kernel(x, cos, sin, Wq, Wk, Wv, Wo, q_gamma, k_gamma, **run_kwargs):
    global LAST_RESULTS
    args = [np.asarray(a, dtype=np.float32)
            for a in (x, cos, sin, Wq, Wk, Wv, Wo, q_gamma, k_gamma)]
    nc = _get_module()
    in_maps = _pack_inputs(*args)
    res = run_bass_kernel_spmd(nc, in_maps, core_ids=list(range(8)), **run_kwargs)
    LAST_RESULTS = res
    y = np.empty((4, T, D), dtype=np.float32)
    for b in range(4):
        y[b] = np.asarray(res.results[2 * b]["y"]) + np.asarray(res.results[2 * b + 1]["y"])
    return y


# revision 35
# speedup vs baseline: 1.0755x; 1.0389x over previous
"""Trainium2 Bass kernel for GQA attention block (nn_Attention_6219112644965).

Reference computation (per batch b):
  q = rope(rmsnorm(x @ Wq, q_gamma), cos, sin)   # 16 heads x 128
  k = rope(rmsnorm(x @ Wk, k_gamma), cos, sin)   # 8 kv heads x 128
  v = x @ Wv
  o = softmax(q k^T / sqrt(128)) v               # GQA: q head h uses kv head h//2
  y = o @ Wo

Sharding: 8 cores = 4 batches x 2 head-groups. Core (b, hg) computes q-heads
[hg*8, hg*8+8) / kv-heads [hg*4, hg*4+4) for batch b over the full sequence and
produces a PARTIAL output y_partial = o_hg @ Wo[hg rows]; the host sums the two
partials per batch. No collectives; per-core FLOPs are exactly total/8.

On-chip layout trick: everything is computed head-dim-major (q^T, k^T:
[HD=128 partitions, T free]) so that NO transposes are needed anywhere:
  q^T[h]   = Wq_h^T x^T          (lhsT = Wq slab,  rhs = x^T)
  k^T[kv]  = Wk_kv^T x^T
  v[t,:]   = x^T^T Wv            (lhsT = x^T slice, rhs = Wv)
  s^T      = k_tile^T q^T        (scores transposed: [Tk part, Tq free])
  o^T      = v_tile^T p^T        (accumulated over Tk tiles in PSUM)
  y        = o^T^T Wo            (lhsT = o^T slice, rhs = Wo rows)
Softmax runs over the PARTITION axis of s^T: exp on ScalarE, column sums via
elementwise accumulation (VectorE) + a ones-matmul partition reduction,
reciprocal via exp(-ln(x)) on ScalarE (one table set for the whole kernel),
partition-broadcast on GpSimd. RMSNorm+RoPE run in the head-dim-major layout
with the head dim PERMUTED (even components first) so the rope pair-shuffle
becomes a 64-partition half swap; gamma and the rope sign are folded into
host-precomputed cos/sin tables, and the rmsnorm scale (a per-t row) commutes
with rope and is applied once at the end. softmax max-subtraction is skipped
(scores are O(5) for rmsnorm-ed q,k; fp32 exp is exact there).
"""
import sys

sys.path.insert(0, "/opt/trn_rl_repo")

from contextlib import ExitStack

import ml_dtypes
import numpy as np

import bass_rust
import concourse.bass as bass
import concourse.mybir as mybir
import concourse.tile as tile
from concourse import bacc, hw_specs
from concourse.bass_utils import run_bass_kernel_spmd

F32 = mybir.dt.float32
BF16 = mybir.dt.bfloat16
AF = mybir.ActivationFunctionType

T = 2048          # sequence length
D = 2048          # model dim
HD = 128          # head dim
NQH = 8           # q heads per core
NKV = 4           # kv heads per core
ND = D // 128     # 16 d-tiles
NTT = T // 128    # 16 t-tiles
EPS = 1e-6

_CACHE = {}
LAST_RESULTS = None


class _Bacc(bacc.Bacc):
    """Bacc with Exp pinned to the natural_log_exp_and_others ACT table set.

    The default static func->set assignment maps Exp to `exp_and_others`
    and Ln to `natural_log_exp_and_others`; a kernel alternating Ln and Exp
    then reloads the ACT tables (~2.7us) on every transition.  Hiding `exp`
    from the other sets makes both resolve to the shared set, so the table
    is loaded once for the whole kernel.
    """

    def insert_act_table_loads(self):
        has_activation = any(
            isinstance(i, mybir.InstActivation)
            for b in self.main_func.blocks
            for i in b.instructions
        )
        if not has_activation:
            return
        tables = []
        for name, funcs in hw_specs.get_activation_tables(self.m.arch).items():
            if name != "natural_log_exp_and_others":
                funcs = funcs - {AF.Exp}
            tables.append((name, funcs))
        bass_rust.insert_act_table_loads(self, tables)


def build_module():
    """Build the per-core Bass program (identical on all 8 cores)."""
    nc = _Bacc("TRN2", target_bir_lowering=False, debug=False)

    # ---- DRAM I/O (host-packed so every DMA is contiguous) ----
    xt_d = nc.dram_tensor("xt", [128, ND, T], BF16, kind="ExternalInput")
    wq_d = nc.dram_tensor("wq", [NQH, 128, ND, HD], BF16, kind="ExternalInput")
    wk_d = nc.dram_tensor("wk", [NKV, 128, ND, HD], BF16, kind="ExternalInput")
    wv_d = nc.dram_tensor("wv", [128, ND, NKV * HD], BF16, kind="ExternalInput")
    wo_d = nc.dram_tensor("wo", [128, NQH, D], BF16, kind="ExternalInput")
    cosq_d = nc.dram_tensor("cosq", [128, T], BF16, kind="ExternalInput")
    sinq_d = nc.dram_tensor("sinq", [128, T], BF16, kind="ExternalInput")
    cosk_d = nc.dram_tensor("cosk", [128, T], BF16, kind="ExternalInput")
    sink_d = nc.dram_tensor("sink", [128, T], BF16, kind="ExternalInput")
    y_d = nc.dram_tensor("y", [T, D], F32, kind="ExternalOutput")

    with tile.TileContext(nc) as tc, ExitStack() as top:
        persist = top.enter_context(tc.tile_pool(name="persist", bufs=1))
        qT = persist.tile([128, NQH, T], BF16, tag="qT")     # q^T, rope+norm done
        kT = persist.tile([128, NKV, T], BF16, tag="kT")     # k^T, rope+norm done
        v_sb = persist.tile([128, NTT, NKV * HD], BF16, tag="v")  # v natural
        ones_col = persist.tile([128, 1], BF16, tag="ones")
        nc.vector.memset(ones_col, 1.0)
        zero128 = persist.tile([128, 1], F32, tag="zero128")
        nc.vector.memset(zero128, 0.0)
        zero1 = zero128[0:1, :]
        epsq1 = persist.tile([1, 1], F32, tag="epsq")
        nc.vector.memset(epsq1, float(HD * EPS))
        epsk1 = persist.tile([1, 1], F32, tag="epsk")
        nc.vector.memset(epsk1, float(EPS))

        # ================= phase 1: projections =================
        with ExitStack() as ph1:
            p1 = ph1.enter_context(tc.tile_pool(name="p1", bufs=1))
            xt = p1.tile([128, ND, T], BF16, tag="xt")
            wv_sb = p1.tile([128, ND, NKV * HD], BF16, tag="wv")
            cosq = p1.tile([128, T], BF16, tag="cosq")
            sinq = p1.tile([128, T], BF16, tag="sinq")
            cosk = p1.tile([128, T], BF16, tag="cosk")
            sink = p1.tile([128, T], BF16, tag="sink")

            wslab_p = ph1.enter_context(tc.tile_pool(name="wslab", bufs=2))
            work = ph1.enter_context(tc.tile_pool(name="pwork", bufs=2))
            mwork = ph1.enter_context(tc.tile_pool(name="mwork", bufs=3))
            ps_mm = ph1.enter_context(tc.tile_pool(name="ps_mm", bufs=4, space="PSUM"))
            ps_row = ph1.enter_context(tc.tile_pool(name="ps_row", bufs=2, space="PSUM"))

            def load_wslab(h, w_dram):
                wsl = wslab_p.tile([128, ND, HD], BF16, tag="wsl")
                nc.sync.dma_start(out=wsl[:], in_=w_dram[h])
                return wsl

            wsl_first = wslab_p.tile([128, ND, HD], BF16, tag="wsl")
            for q4 in range(4):
                nc.sync.dma_start(out=wsl_first[:, 4 * q4:4 * (q4 + 1), :],
                                  in_=wk_d[0, :, 4 * q4:4 * (q4 + 1), :])
            for d in range(ND):
                nc.sync.dma_start(out=xt[:, d, :], in_=xt_d[:, d, :])
            nc.sync.dma_start(out=cosk[:], in_=cosk_d[:])
            nc.sync.dma_start(out=sink[:], in_=sink_d[:])
            nc.sync.dma_start(out=cosq[:], in_=cosq_d[:])
            nc.sync.dma_start(out=sinq[:], in_=sinq_d[:])
            nc.sync.dma_start(out=wv_sb[:], in_=wv_d[:])

            def qk_proj(h, w_dram, out_T, cos_t, sin_t, is_q, wsl=None):
                """One head's projection + rmsnorm + rope, head-dim-major.

                Chunks are processed in pairs sharing each stationary
                LDWEIGHTS (two matmuls per weight tile)."""
                if wsl is None:
                    wsl = load_wslab(h, w_dram)
                pair_ps = []
                for c in range(T // 512):
                    cs = slice(c * 512, (c + 1) * 512)
                    if c % 2 == 0:
                        ps_a = ps_mm.tile([128, 512], F32, tag="mm")
                        ps_b = ps_mm.tile([128, 512], F32, tag="mm")
                        for d in range(ND):
                            nc.tensor.matmul(ps_a, wsl[:, d, :], xt[:, d, cs],
                                             start=(d == 0), stop=(d == ND - 1))
                            nc.tensor.matmul(ps_b, wsl[:, d, :],
                                             xt[:, d, cs.start + 512:cs.stop + 512],
                                             start=(d == 0), stop=(d == ND - 1))
                        pair_ps = [ps_a, ps_b]
                    raw_ps = pair_ps[c % 2]
                    raw_bf = work.tile([128, 512], BF16, tag="raw")
                    nc.scalar.copy(raw_bf, raw_ps)
                    # sum of squares over head dim (partitions) via ones-matmul
                    sq = work.tile([128, 512], BF16, tag="sq")
                    nc.scalar.activation(out=sq, in_=raw_ps, func=AF.Square,
                                         bias=zero128[:, :])
                    ssq = ps_row.tile([1, 512], F32, tag="row")
                    nc.tensor.matmul(ssq, ones_col, sq, start=True, stop=True)
                    # rec = rsqrt(mean+eps) (k) or rsqrt(mean+eps)/sqrt(HD) (q),
                    # via exp(-0.5*ln(scale*ssq+bias)) -- one ACT table set.
                    lnrow = work.tile([1, 512], F32, tag="lnrow")
                    if is_q:
                        nc.scalar.activation(out=lnrow, in_=ssq, func=AF.Ln,
                                             scale=1.0, bias=epsq1[:, :])
                    else:
                        nc.scalar.activation(out=lnrow, in_=ssq, func=AF.Ln,
                                             scale=1.0 / HD, bias=epsk1[:, :])
                    rec = work.tile([1, 512], F32, tag="recrow")
                    nc.scalar.activation(out=rec, in_=lnrow, func=AF.Exp, scale=-0.5,
                                         bias=zero1)
                    bc = work.tile([128, 512], F32, tag="bc")
                    nc.gpsimd.partition_broadcast(bc, rec)
                    # rope: out = raw*cos + swap64(raw)*sin  (gamma folded in tables)
                    m1 = mwork.tile([128, 512], BF16, tag="m1")
                    nc.vector.tensor_mul(m1, raw_bf, cos_t[:, cs])
                    swp = mwork.tile([128, 512], BF16, tag="swp")
                    nc.vector.tensor_copy(swp[0:64, :], raw_bf[64:128, :])
                    nc.vector.tensor_copy(swp[64:128, :], raw_bf[0:64, :])
                    m2 = mwork.tile([128, 512], BF16, tag="m2")
                    nc.vector.tensor_mul(m2, swp, sin_t[:, cs])
                    m3 = mwork.tile([128, 512], BF16, tag="m3")
                    nc.vector.tensor_add(m3, m1, m2)
                    nc.vector.tensor_mul(out_T[:, h, cs], m3, bc)

            for kv in range(NKV):
                qk_proj(kv, wk_d, kT, cosk, sink, is_q=False,
                        wsl=wsl_first if kv == 0 else None)

            for tt in range(NTT):
                v_ps = ps_mm.tile([128, 512], F32, tag="mm")
                ts_ = slice(tt * 128, (tt + 1) * 128)
                for d in range(ND):
                    nc.tensor.matmul(v_ps, xt[:, d, ts_], wv_sb[:, d, :],
                                     start=(d == 0), stop=(d == ND - 1))
                nc.scalar.copy(v_sb[:, tt, :], v_ps)

            for h in range(NQH):
                qk_proj(h, wq_d, qT, cosq, sinq, is_q=True)

        # ================= phase 2: attention + out-projection =================
        with ExitStack() as ph2:
            main2 = ph2.enter_context(tc.tile_pool(name="main2", bufs=1))
            oT = main2.tile([128, NQH, T], BF16, tag="oT")
            wo_sb = main2.tile([128, NQH, D], BF16, tag="wo")
            nc.sync.dma_start(out=wo_sb[:], in_=wo_d[:])

            ysb_p = ph2.enter_context(tc.tile_pool(name="ysb", bufs=4))
            ps_y = ph2.enter_context(tc.tile_pool(name="ps_y", bufs=2, space="PSUM"))
            awork = ph2.enter_context(tc.tile_pool(name="awork", bufs=2))
            pwork = ph2.enter_context(tc.tile_pool(name="ppool", bufs=5))
            ps_s = ph2.enter_context(tc.tile_pool(name="ps_s", bufs=2, space="PSUM"))
            ps_o = ph2.enter_context(tc.tile_pool(name="ps_o", bufs=1, space="PSUM"))

            TQC = 1024  # tq chunk

            def attn(h, c):
                kv = h // 2
                cs0 = slice(c * TQC, c * TQC + 512)
                cs1 = slice(c * TQC + 512, (c + 1) * TQC)
                o_ps = ps_o.tile([128, TQC], F32, tag="o")
                colsum = awork.tile([128, TQC], BF16, tag="colsum")
                for tk in range(NTT):
                    ks = slice(tk * 128, (tk + 1) * 128)
                    s_ps = ps_s.tile([128, TQC], F32, tag="s")
                    nc.tensor.matmul(s_ps[:, 0:512], kT[:, kv, ks], qT[:, h, cs0],
                                     start=True, stop=True)
                    nc.tensor.matmul(s_ps[:, 512:TQC], kT[:, kv, ks], qT[:, h, cs1],
                                     start=True, stop=True)
                    p_bf = pwork.tile([128, TQC], BF16, tag="p")
                    nc.scalar.activation(out=p_bf, in_=s_ps, func=AF.Exp,
                                         bias=zero128[:, :])
                    if tk == 0:
                        nc.vector.tensor_copy(colsum, p_bf)
                    else:
                        nc.vector.tensor_add(colsum, colsum, p_bf)
                    vt = v_sb[:, tk, kv * HD:(kv + 1) * HD]
                    nc.tensor.matmul(o_ps[:, 0:512], vt, p_bf[:, 0:512],
                                     start=(tk == 0), stop=(tk == NTT - 1))
                    nc.tensor.matmul(o_ps[:, 512:TQC], vt, p_bf[:, 512:TQC],
                                     start=(tk == 0), stop=(tk == NTT - 1))
                # free the o_ps PSUM slot immediately (unnormalized copy);
                # the denominator chain + normalize run off PE's critical path
                oTun = awork.tile([128, TQC], F32, tag="oTun")
                nc.vector.tensor_copy(oTun, o_ps)
                # denominator rows: in chunk 0 use the idle out-projection PSUM
                # banks so the s pool keeps pure double-buffered rotation (no
                # head-boundary slot steal); in later chunks ps_y is busy with
                # out-projection tiles, so share the s pool there instead.
                recr = awork.tile([1, TQC], F32, tag="recr")
                if c == 0:
                    den_a = ps_y.tile([1, 512], F32, tag="y")
                    den_b = ps_y.tile([1, 512], F32, tag="y")
                    nc.tensor.matmul(den_a, ones_col, colsum[:, 0:512],
                                     start=True, stop=True)
                    nc.tensor.matmul(den_b, ones_col, colsum[:, 512:TQC],
                                     start=True, stop=True)
                    # reciprocal on VectorE (custom uop, ~51 ULP) — keeps the
                    # denominator chain off exp-bound ScalarE entirely
                    nc.vector.reciprocal_approx_fast(out=recr[:, 0:512], in_=den_a)
                    nc.vector.reciprocal_approx_fast(out=recr[:, 512:TQC], in_=den_b)
                else:
                    den = ps_s.tile([128, TQC], F32, tag="s")  # only row 0 used
                    nc.tensor.matmul(den[0:1, 0:512], ones_col, colsum[:, 0:512],
                                     start=True, stop=True)
                    nc.tensor.matmul(den[0:1, 512:TQC], ones_col, colsum[:, 512:TQC],
                                     start=True, stop=True)
                    nc.vector.reciprocal_approx_fast(out=recr, in_=den[0:1, :])
                bc = awork.tile([128, TQC], F32, tag="abc")
                nc.gpsimd.partition_broadcast(bc, recr)
                nc.vector.tensor_mul(oT[:, h, c * TQC:(c + 1) * TQC], oTun, bc)

            def outproj(tt, tail=False):
                ts_ = slice(tt * 128, (tt + 1) * 128)
                y_sb = ysb_p.tile([128, D], F32, tag="ysb")
                for np_ in range(2):  # pairs of 512-col chunks share LDWEIGHTS
                    ns0 = slice(np_ * 1024, np_ * 1024 + 512)
                    ns1 = slice(np_ * 1024 + 512, (np_ + 1) * 1024)
                    y_ps0 = ps_y.tile([128, 512], F32, tag="y")
                    y_ps1 = ps_y.tile([128, 512], F32, tag="y")
                    for h in range(NQH):
                        nc.tensor.matmul(y_ps0, oT[:, h, ts_], wo_sb[:, h, ns0],
                                         start=(h == 0), stop=(h == NQH - 1))
                        nc.tensor.matmul(y_ps1, oT[:, h, ts_], wo_sb[:, h, ns1],
                                         start=(h == 0), stop=(h == NQH - 1))
                    # tail tiles copy on ScalarE (idle there); interleaved ones
                    # on VectorE (ScalarE is exp-bound mid-attention)
                    cp = nc.scalar.copy if tail else nc.vector.tensor_copy
                    cp(y_sb[:, ns0], y_ps0)
                    cp(y_sb[:, ns1], y_ps1)
                if tail:
                    for q4 in range(4):
                        nc.sync.dma_start(out=y_d[ts_, 512 * q4:512 * (q4 + 1)],
                                          in_=y_sb[:, 512 * q4:512 * (q4 + 1)])
                else:
                    nc.sync.dma_start(out=y_d[ts_, 0:1024], in_=y_sb[:, 0:1024])
                    nc.sync.dma_start(out=y_d[ts_, 1024:D], in_=y_sb[:, 1024:D])

            # attention chunk 0, then chunk 1 with chunk-0's out-projection
            # interleaved so its matmuls fill PE slack while ACT runs exps.
            for h in range(NQH):
                attn(h, 0)
            for h in range(NQH):
                attn(h, 1)
                outproj(h)          # chunk-0 rows 0..7
            for tt in range(8, NTT):
                outproj(tt, tail=True)  # chunk-1 rows

    nc.compile()
    return nc


def _get_module():
    if "nc" not in _CACHE:
        _CACHE["nc"] = build_module()
    return _CACHE["nc"]


def _pack_inputs(x, cos, sin, Wq, Wk, Wv, Wo, q_gamma, k_gamma):
    """Host-side prep: per-core input dicts with bf16 packed layouts."""
    bf16 = ml_dtypes.bfloat16
    perm = np.concatenate([np.arange(0, HD, 2), np.arange(1, HD, 2)])  # [128]
    partner = np.concatenate([perm[64:], perm[:64]])                   # gamma idx for sin term
    sign = np.concatenate([-np.ones(64), np.ones(64)]).astype(np.float32)

    cosT = np.ascontiguousarray(cos.T)  # [128, T]
    sinT = np.ascontiguousarray(sin.T)

    def tables(gamma):
        c = (cosT[perm] * gamma[perm][:, None]).astype(bf16)
        s = (sinT[perm] * sign[:, None] * gamma[partner][:, None]).astype(bf16)
        return np.ascontiguousarray(c), np.ascontiguousarray(s)

    cosq, sinq = tables(q_gamma.astype(np.float32))
    cosk, sink = tables(k_gamma.astype(np.float32))

    per_hg = []
    for hg in range(2):
        qh = slice(hg * NQH * HD, (hg + 1) * NQH * HD)
        kh = slice(hg * NKV * HD, (hg + 1) * NKV * HD)
        wq = Wq[:, qh].reshape(ND, 128, NQH, HD)[..., perm]
        wq = np.ascontiguousarray(wq.transpose(2, 1, 0, 3)).astype(bf16)
        wk = Wk[:, kh].reshape(ND, 128, NKV, HD)[..., perm]
        wk = np.ascontiguousarray(wk.transpose(2, 1, 0, 3)).astype(bf16)
        wv = Wv[:, kh].reshape(ND, 128, NKV * HD)
        wv = np.ascontiguousarray(wv.transpose(1, 0, 2)).astype(bf16)
        wo = Wo[hg * NQH * HD:(hg + 1) * NQH * HD, :].reshape(NQH, 128, D)
        wo = np.ascontiguousarray(wo.transpose(1, 0, 2)).astype(bf16)
        per_hg.append(dict(wq=wq, wk=wk, wv=wv, wo=wo))

    in_maps = []
    for b in range(4):
        xt = np.ascontiguousarray(
            x[b].T.reshape(ND, 128, T).transpose(1, 0, 2)).astype(bf16)
        for hg in range(2):
            m = dict(xt=xt, cosq=cosq, sinq=sinq, cosk=cosk, sink=sink,
                     **per_hg[hg])
            in_maps.append(m)
    return in_maps


def kernel(x, cos, sin, Wq, Wk, Wv, Wo, q_gamma, k_gamma, **run_kwargs):
    global LAST_RESULTS
    args = [np.asarray(a, dtype=np.float32)
            for a in (x, cos, sin, Wq, Wk, Wv, Wo, q_gamma, k_gamma)]
    nc = _get_module()
    in_maps = _pack_inputs(*args)
    res = run_bass_kernel_spmd(nc, in_maps, core_ids=list(range(8)), **run_kwargs)
    LAST_RESULTS = res
    y = np.empty((4, T, D), dtype=np.float32)
    for b in range(4):
        y[b] = np.asarray(res.results[2 * b]["y"]) + np.asarray(res.results[2 * b + 1]["y"])
    return y
